# revision 19
# baseline (speedup 1.0000x reference)
"""Two-layer GRU encoder (B=64, T=12, N=325, D=2, H=256) on 8 TRN2 NeuronCores.

v3: PE-lean revision of the v2.5 pipeline.
 - K=3 x-projections and K=1 L1-bias matmuls run as row-tiled concurrent
   groups (tile_position strips 0/32/64/96), cutting 12 full-width PE passes
   per stage-pair down to 4 group passes.  The moving x rows (+ones) are
   replicated across the four strips of one SBUF tile.
 - Wx1 candidate projection moved to fp8 DoubleRow (h0's e4m3 copy already
   exists for the zr path), 4 fp16 passes -> 2 DR passes.
 - Hidden/work tiles are tightly packed ([128, 2*mw], halves adjacent) so
   every DVE tensor_tensor runs as a flat single-range 2x op.  The fp8 h
   copies keep the 512-stride layout DoubleRow requires.

Pipeline structure (SKEW=3 over (t, layer, chunk) stages with two 4-bank
PSUM regions, cand borrowing its region's z banks) is unchanged from v2.5.
"""

import numpy as np
import ml_dtypes
from contextlib import ExitStack

import concourse.bass as bass
import concourse.tile as tile
from concourse import bacc, mybir
from concourse import bass_utils

F16 = np.float16
E4M3 = ml_dtypes.float8_e4m3fn
AF = mybir.ActivationFunctionType
DT = mybir.dt
DR = mybir.MatmulPerfMode.DoubleRow

H = 256
T = 12
B = 64
N = 325
D = 2
NCORES = 8
B_SH = B // NCORES            # 8
M = B_SH * N                  # 2600
_CWS = [434, 434, 434, 434, 432, 432]
CHUNKS = []
_o = 0
for _w in _CWS:
    CHUNKS.append((_o, _w))
    _o += _w
NCH = len(CHUNKS)
MWMAX = 434
OUTW = 2 * M                  # 5200 = sum of 2*mw
SKEW = 3
STRIPS = (0, 32, 64, 96)

_CACHE = {}


def _build_nc():
    nc = bacc.Bacc("TRN2", target_bir_lowering=False, debug=False,
                   enable_asserts=False)
    f16 = DT.float16
    f8 = DT.float8e4
    f32 = DT.float32

    xt_d = nc.dram_tensor("xt", (3, T * M), f16, kind="ExternalInput").ap()
    wx0p_d = nc.dram_tensor("wx0p", (99, 256), f16, kind="ExternalInput").ap()
    bias1p_d = nc.dram_tensor("bias1p", (99, 256), f16, kind="ExternalInput").ap()
    whzr0_d = nc.dram_tensor("whzr0", (128, 1024), f8, kind="ExternalInput").ap()
    whh0_d = nc.dram_tensor("whh0", (128, 512), f8, kind="ExternalInput").ap()
    wx1zr_d = nc.dram_tensor("wx1zr", (128, 1024), f8, kind="ExternalInput").ap()
    wx1c_d = nc.dram_tensor("wx1c", (128, 512), f8, kind="ExternalInput").ap()
    whzr1_d = nc.dram_tensor("whzr1", (128, 1024), f8, kind="ExternalInput").ap()
    whh1_d = nc.dram_tensor("whh1", (128, 512), f8, kind="ExternalInput").ap()
    out_d = nc.dram_tensor("out", (2, 128, OUTW), f16,
                           kind="ExternalOutput").ap()

    with tile.TileContext(nc) as tc, ExitStack() as ctx:
        const = ctx.enter_context(tc.tile_pool(name="const", bufs=1))
        hpool = ctx.enter_context(tc.tile_pool(name="hstate", bufs=1))
        work = ctx.enter_context(tc.tile_pool(name="work", bufs=4))
        psum = ctx.enter_context(tc.tile_pool(name="psum", bufs=1, space="PSUM"))

        def load(name, dram, shape, dtype):
            t_ = const.tile(list(shape), dtype, tag=name, name=name)
            nc.sync.dma_start(t_[:], dram[:])
            return t_

        # DMA order matters for the pipeline ramp: t=0 needs wx0p + the first
        # x chunks + L1 weights; whzr*/whh* only matter from t=1.
        wx0p = load("wx0p", wx0p_d, (99, 256), f16)
        bias1p = load("bias1p", bias1p_d, (99, 256), f16)
        # x rows (x0, x1, ones) replicated on the four 32-partition strips
        xtr = const.tile([99, T * M], f16, tag="xtr", name="xtr")
        for g in range(4):
            nc.sync.dma_start(xtr[STRIPS[g]:STRIPS[g] + 3, 0:M],
                              xt_d[:, 0:M])
        ones4 = const.tile([97, MWMAX], f16, tag="ones4", name="ones4")
        nc.vector.memset(ones4[:], 1.0)
        wx1zr = load("wx1zr", wx1zr_d, (128, 1024), f8)
        wx1c = load("wx1c", wx1c_d, (128, 512), f8)
        whzr0 = load("whzr0", whzr0_d, (128, 1024), f8)
        whzr1 = load("whzr1", whzr1_d, (128, 1024), f8)
        whh0 = load("whh0", whh0_d, (128, 512), f8)
        whh1 = load("whh1", whh1_d, (128, 512), f8)
        for g in range(4):
            nc.sync.dma_start(xtr[STRIPS[g]:STRIPS[g] + 3, M:T * M],
                              xt_d[:, M:T * M])

        # single 8-bank PSUM tile, manually banked
        pp8 = psum.tile([128, 8, 512], f32, tag="pp8", name="pp8", bufs=1)

        # HAM warmup: dense K=1 matmul burst during the initial DMA wait so
        # the PE clock is at 8/8 before the first real stage.  Banks are
        # reset by each stage's start=True matmuls, so garbage is harmless.
        for wi in range(16):
            nc.tensor.matmul(pp8[:, wi % 8, 0:MWMAX], ones4[0:1, 0:128],
                             ones4[0:1, 0:MWMAX], start=True, stop=True)

        # fp16 hidden states, halves adjacent at [0:mw] and [mw:2mw]
        hst = {}
        h8st = {}
        for L in (0, 1):
            for ci in range(NCH):
                for pp in (0, 1):
                    nm = f"h{L}_{ci}_{pp}"
                    hst[(L, ci, pp)] = hpool.tile([128, 2 * MWMAX], f16,
                                                  tag=nm, name=nm)
                    nm8 = f"h8_{L}_{ci}_{pp}"
                    h8st[(L, ci, pp)] = hpool.tile([128, 1024], f8,
                                                   tag=nm8, name=nm8)

        def h8_v(tile_, mw):  # [128, 2, mw] packed e4m3 view (stride 512)
            return tile_[:, :].rearrange("p (k m) -> p k m", k=2)[:, :, 0:mw]

        def wdr(w, g):  # [128, 2, 128] DR weight view for gate-half g
            return w[:, g * 256:(g + 1) * 256].rearrange("p (k f) -> p k f", k=2)

        # weight gate order: cols [z | r]; banks in region: [za zb ra rb]
        # r-banks first (they only wait sigma(i-2)); z-banks last (they also
        # wait the tanh of the cand stage that borrowed them one slot ago).
        BORDER = (2, 3, 0, 1)

        def emit_zr(nc_, t, L, ci, reg, s_zr):
            m0, mw = CHUNKS[ci]
            first = t == 0
            pp_r = 1 - t % 2

            def dr_block(w, h8t, start):
                for g in BORDER:
                    nc_.tensor.matmul(pp8[:, reg + g, 0:mw], wdr(w, g),
                                      h8_v(h8t, mw), start=start, stop=False,
                                      perf_mode=DR)

            if L == 0:
                if not first:
                    dr_block(whzr0, h8st[(0, ci, pp_r)], True)
                # row-tiled K=3 x-projection: strip 32g -> bank reg+g
                for g in BORDER:
                    sp = STRIPS[g]
                    nc_.tensor.matmul(
                        pp8[:, reg + g, 0:mw],
                        wx0p[sp:sp + 3, 0:128],
                        xtr[sp:sp + 3, t * M + m0: t * M + m0 + mw],
                        start=first, stop=True, tile_position=(sp, 0))
            else:
                h08n = h8st[(0, ci, t % 2)]
                if not first:
                    dr_block(whzr1, h8st[(1, ci, pp_r)], True)
                dr_block(wx1zr, h08n, first)
                # row-tiled K=1 bias: strip 32g -> bank reg+g
                for g in BORDER:
                    sp = STRIPS[g]
                    nc_.tensor.matmul(
                        pp8[:, reg + g, 0:mw],
                        bias1p[sp:sp + 1, 0:128],
                        ones4[sp:sp + 1, 0:mw],
                        start=False, stop=True, tile_position=(sp, 0))
            # fused sigmoid over the region's 4 banks -> s_zr [za zb ra rb]
            nc_.scalar.activation(
                s_zr[:, 0:4 * mw].rearrange("p (g m) -> p g m", g=4),
                pp8[:, reg:reg + 4, 0:mw], AF.Sigmoid)

        def emit_cand(nc_, t, L, ci, reg, s_zr, c):
            # c banks = z-banks (reg+0, reg+1) of this stage's own region
            m0, mw = CHUNKS[ci]
            first = t == 0
            pp_r = 1 - t % 2
            pp_w = t % 2
            hp = hst[(L, ci, pp_r)]
            hn = hst[(L, ci, pp_w)]
            rh8 = None
            if not first:
                # r*h written directly as the packed e4m3 DoubleRow moving
                # operand for whh
                rh8 = work.tile([128, 1024], DT.float8e4, tag="rh8",
                                name=f"rh8{L}{ci}")
                nc_.vector.tensor_mul(
                    h8_v(rh8, mw),
                    s_zr[:, 2 * mw:4 * mw].rearrange("p (k m) -> p k m", k=2),
                    hp[:, 0:2 * mw].rearrange("p (k m) -> p k m", k=2))
            whh = whh0 if L == 0 else whh1
            if L == 0:
                # row-tiled K=3 cand x-projection: strips 0,32 -> banks reg+0,1
                for g in range(2):
                    sp = STRIPS[g]
                    nc_.tensor.matmul(
                        pp8[:, reg + g, 0:mw],
                        wx0p[sp:sp + 3, 128:256],
                        xtr[sp:sp + 3, t * M + m0: t * M + m0 + mw],
                        start=True, stop=first, tile_position=(sp, 0))
            else:
                # row-tiled K=1 cand bias (start) + Wx1c fp8 DR on h0
                for g in range(2):
                    sp = STRIPS[g]
                    nc_.tensor.matmul(
                        pp8[:, reg + g, 0:mw],
                        bias1p[sp:sp + 1, 128:256],
                        ones4[sp:sp + 1, 0:mw],
                        start=True, stop=False, tile_position=(sp, 0))
                h08n = h8st[(0, ci, pp_w)]
                for g in range(2):
                    nc_.tensor.matmul(pp8[:, reg + g, 0:mw], wdr(wx1c, g),
                                      h8_v(h08n, mw), start=False, stop=first,
                                      perf_mode=DR)
            if not first:
                for g in range(2):
                    nc_.tensor.matmul(pp8[:, reg + g, 0:mw], wdr(whh, g),
                                      h8_v(rh8, mw), start=False,
                                      stop=(g == 1), perf_mode=DR)
            # fused tanh over the 2 borrowed banks
            nc_.scalar.activation(
                c[:, 0:2 * mw].rearrange("p (g m) -> p g m", g=2),
                pp8[:, reg:reg + 2, 0:mw], AF.Tanh)
            # blend: hn = hp + z*(c - hp)
            s_z = s_zr[:, 0:2 * mw]
            if first:
                nc_.vector.tensor_mul(hn[:, 0:2 * mw], s_z, c[:, 0:2 * mw])
            else:
                d = work.tile([128, 2 * MWMAX], DT.float16, tag="d",
                              name=f"d{L}{ci}")
                nc_.vector.tensor_sub(d[:, 0:2 * mw], c[:, 0:2 * mw],
                                      hp[:, 0:2 * mw])
                zd = work.tile([128, 2 * MWMAX], DT.float16, tag="zd",
                               name=f"zd{L}{ci}")
                nc_.vector.tensor_mul(zd[:, 0:2 * mw], s_z, d[:, 0:2 * mw])
                nc_.vector.tensor_add(hn[:, 0:2 * mw], hp[:, 0:2 * mw],
                                      zd[:, 0:2 * mw])
            # packed e4m3 copy for next-step DR reads (and L1 xp for L==0);
            # dead at the last step for L==1
            if L == 0 or t < T - 1:
                # SWDGE cast-DMA keeps the e4m3 repack off the DVE; consumers
                # are >=6 stages away so the ~1us DMA latency is hidden.
                h8n = h8st[(L, ci, pp_w)]
                nc_.gpsimd.dma_start(
                    h8_v(h8n, mw),
                    hn[:, 0:2 * mw].rearrange("p (k m) -> p k m", k=2))

        stages = [(t, L, ci) for t in range(T) for L in (0, 1)
                  for ci in range(NCH)]
        pending = {}
        for si in range(len(stages) + SKEW):
            if si < len(stages):
                t, L, ci = stages[si]
                s_zr = work.tile([128, 4 * MWMAX], DT.float16, tag="szr",
                                 name=f"szr{L}{t}{ci}", bufs=SKEW + 2)
                emit_zr(nc, t, L, ci, 4 * (si % 2), s_zr)
                pending[si] = s_zr
            if si >= SKEW:
                sj = si - SKEW
                t, L, cj = stages[sj]
                c = work.tile([128, 2 * MWMAX], DT.float16, tag="c",
                              name=f"c{L}{t}{cj}")
                emit_cand(nc, t, L, cj, 4 * (sj % 2), pending.pop(sj), c)

        ppf = (T - 1) % 2
        for L in (0, 1):
            for ci, (m0, mw) in enumerate(CHUNKS):
                nc.sync.dma_start(out_d[L, :, 2 * m0:2 * m0 + 2 * mw],
                                  hst[(L, ci, ppf)][:, 0:2 * mw])

    nc.compile()
    return nc


def _prep_weights(inputs):
    def f32(x):
        return np.asarray(x, np.float32)

    def q8c(x):
        return np.clip(f32(x), -240, 240).astype(E4M3)

    def dr_pack(W):  # (256, G*128) -> (128, G*256) DR layout
        G = W.shape[1] // 128
        out = np.zeros((128, G * 256), np.float32)
        for g in range(G):
            for k in range(2):
                out[:, g * 256 + k * 128:g * 256 + (k + 1) * 128] = \
                    W[k * 128:(k + 1) * 128, g * 128:(g + 1) * 128]
        return out

    def kstack(W):  # (256, C) -> (128, 2C)
        return np.concatenate([W[:128], W[128:]], axis=1)

    ball = {}
    for L in (0, 1):
        bx = f32(inputs[f"bx{L}"])
        bhzr = f32(inputs[f"bhzr{L}"])
        bhh = f32(inputs[f"bhh{L}"])
        ball[L] = np.concatenate([bx[:2 * H] + bhzr, bx[2 * H:] + bhh])

    # wx0p: strips of [Wx0 | b0] columns; [sp:sp+3, 0:128] = zr gate g,
    # [sp:sp+3, 128:256] = cand gate g (g=0,1)
    wx0 = np.concatenate([f32(inputs["Wx0"]), ball[0][None, :]], axis=0)
    wx0p = np.zeros((99, 256), np.float32)
    bias1p = np.zeros((99, 256), np.float32)
    for g in range(4):
        sp = STRIPS[g]
        wx0p[sp:sp + 3, 0:128] = wx0[:, g * 128:(g + 1) * 128]
        bias1p[sp, 0:128] = ball[1][g * 128:(g + 1) * 128]
    for g in range(2):
        sp = STRIPS[g]
        wx0p[sp:sp + 3, 128:256] = wx0[:, 512 + g * 128:512 + (g + 1) * 128]
        bias1p[sp, 128:256] = ball[1][512 + g * 128:512 + (g + 1) * 128]

    wx1 = f32(inputs["Wx1"])
    return {
        "wx0p": wx0p.astype(F16),
        "bias1p": bias1p.astype(F16),
        "whzr0": q8c(dr_pack(f32(inputs["Whzr0"]))),
        "whh0": q8c(dr_pack(f32(inputs["Whh0"]))),
        "wx1zr": q8c(dr_pack(wx1[:, :2 * H])),
        "wx1c": q8c(dr_pack(wx1[:, 2 * H:])),
        "whzr1": q8c(dr_pack(f32(inputs["Whzr1"]))),
        "whh1": q8c(dr_pack(f32(inputs["Whh1"]))),
    }


def kernel(**inputs):
    X = np.asarray(inputs["X"], np.float32)
    shared = _prep_weights(inputs)

    if "nc" not in _CACHE:
        _CACHE["nc"] = _build_nc()
    nc = _CACHE["nc"]

    in_maps = []
    ones = np.ones((1, T * M), np.float32)
    for c in range(NCORES):
        Xc = X[c * B_SH:(c + 1) * B_SH]                      # (8, T, N, D)
        xt = np.ascontiguousarray(Xc.transpose(3, 1, 0, 2)).reshape(D, T * M)
        m = dict(shared)
        m["xt"] = np.concatenate([xt, ones], axis=0).astype(F16)
        in_maps.append(m)
    _CACHE["in_maps"] = in_maps

    res = None
    for attempt in range(3):
        try:
            res = bass_utils.run_bass_kernel_spmd(nc, in_maps,
                                                  core_ids=list(range(NCORES)))
            break
        except Exception:
            if attempt == 2:
                raise
    assert res is not None

    out = np.empty((2, B, N, H), np.float32)
    for c in range(NCORES):
        arr = np.asarray(res.results[c]["out"], dtype=np.float32)  # (2,128,OUTW)
        per_core = np.empty((2, M, H), np.float32)
        for ci, (m0, mw) in enumerate(CHUNKS):
            blk = arr[:, :, 2 * m0:2 * m0 + 2 * mw].reshape(2, 128, 2, mw)
            per_core[:, m0:m0 + mw, :] = blk.transpose(0, 3, 2, 1).reshape(2, mw, H)
        out[:, c * B_SH:(c + 1) * B_SH] = per_core.reshape(2, B_SH, N, H)
    return out


# revision 25
# speedup vs baseline: 1.1077x; 1.1077x over previous
"""Two-layer GRU encoder (B=64, T=12, N=325, D=2, H=256) on 8 TRN2 NeuronCores.

v3: PE-lean revision of the v2.5 pipeline.
 - K=3 x-projections and K=1 L1-bias matmuls run as row-tiled concurrent
   groups (tile_position strips 0/32/64/96), cutting 12 full-width PE passes
   per stage-pair down to 4 group passes.  The moving x rows (+ones) are
   replicated across the four strips of one SBUF tile.
 - Wx1 candidate projection moved to fp8 DoubleRow (h0's e4m3 copy already
   exists for the zr path), 4 fp16 passes -> 2 DR passes.
 - Hidden/work tiles are tightly packed ([128, 2*mw], halves adjacent) so
   every DVE tensor_tensor runs as a flat single-range 2x op.  The fp8 h
   copies keep the 512-stride layout DoubleRow requires.

Pipeline structure (SKEW=3 over (t, layer, chunk) stages with two 4-bank
PSUM regions, cand borrowing its region's z banks) is unchanged from v2.5.
"""

import numpy as np
import ml_dtypes
from contextlib import ExitStack

import concourse.bass as bass
import concourse.tile as tile
from concourse import bacc, mybir
from concourse import bass_utils

F16 = np.float16
E4M3 = ml_dtypes.float8_e4m3fn
AF = mybir.ActivationFunctionType
DT = mybir.dt
DR = mybir.MatmulPerfMode.DoubleRow

H = 256
T = 12
B = 64
N = 325
D = 2
NCORES = 8
B_SH = B // NCORES            # 8
M = B_SH * N                  # 2600
_CWS = [434, 434, 434, 434, 432, 432]
CHUNKS = []
_o = 0
for _w in _CWS:
    CHUNKS.append((_o, _w))
    _o += _w
NCH = len(CHUNKS)
MWMAX = 434
OUTW = 2 * M                  # 5200 = sum of 2*mw
SKEW = 3
STRIPS = (0, 32, 64, 96)

_CACHE = {}


def _build_nc():
    nc = bacc.Bacc("TRN2", target_bir_lowering=False, debug=False,
                   enable_asserts=False)
    f16 = DT.float16
    f8 = DT.float8e4
    f32 = DT.float32

    xt_d = nc.dram_tensor("xt", (3, T * M), f16, kind="ExternalInput").ap()
    wx0p_d = nc.dram_tensor("wx0p", (99, 256), f16, kind="ExternalInput").ap()
    bias1p_d = nc.dram_tensor("bias1p", (99, 256), f16, kind="ExternalInput").ap()
    whzr0_d = nc.dram_tensor("whzr0", (128, 1024), f8, kind="ExternalInput").ap()
    whh0_d = nc.dram_tensor("whh0", (128, 512), f16, kind="ExternalInput").ap()
    wx1zr_d = nc.dram_tensor("wx1zr", (128, 1024), f8, kind="ExternalInput").ap()
    wx1c_d = nc.dram_tensor("wx1c", (128, 512), f8, kind="ExternalInput").ap()
    whzr1_d = nc.dram_tensor("whzr1", (128, 1024), f8, kind="ExternalInput").ap()
    whh1_d = nc.dram_tensor("whh1", (128, 512), f16, kind="ExternalInput").ap()
    out_d = nc.dram_tensor("out", (2, 128, OUTW), f16,
                           kind="ExternalOutput").ap()

    with tile.TileContext(nc) as tc, ExitStack() as ctx:
        const = ctx.enter_context(tc.tile_pool(name="const", bufs=1))
        hpool = ctx.enter_context(tc.tile_pool(name="hstate", bufs=1))
        work = ctx.enter_context(tc.tile_pool(name="work", bufs=4))
        psum = ctx.enter_context(tc.tile_pool(name="psum", bufs=1, space="PSUM"))

        def load(name, dram, shape, dtype):
            t_ = const.tile(list(shape), dtype, tag=name, name=name)
            nc.sync.dma_start(t_[:], dram[:])
            return t_

        # DMA order matters for the pipeline ramp: t=0 needs wx0p + the first
        # x chunks + L1 weights; whzr*/whh* only matter from t=1.
        wx0p = load("wx0p", wx0p_d, (99, 256), f16)
        bias1p = load("bias1p", bias1p_d, (99, 256), f16)
        # x rows (x0, x1, ones) replicated on the four 32-partition strips
        xtr = const.tile([99, T * M], f16, tag="xtr", name="xtr")
        for g in range(4):
            nc.sync.dma_start(xtr[STRIPS[g]:STRIPS[g] + 3, 0:M],
                              xt_d[:, 0:M])
        ones4 = const.tile([97, MWMAX], f16, tag="ones4", name="ones4")
        nc.vector.memset(ones4[:], 1.0)
        wx1zr = load("wx1zr", wx1zr_d, (128, 1024), f8)
        wx1c = load("wx1c", wx1c_d, (128, 512), f8)
        whzr0 = load("whzr0", whzr0_d, (128, 1024), f8)
        whzr1 = load("whzr1", whzr1_d, (128, 1024), f8)
        whh0 = load("whh0", whh0_d, (128, 512), f16)
        whh1 = load("whh1", whh1_d, (128, 512), f16)
        for g in range(4):
            nc.sync.dma_start(xtr[STRIPS[g]:STRIPS[g] + 3, M:T * M],
                              xt_d[:, M:T * M])

        # single 8-bank PSUM tile, manually banked
        pp8 = psum.tile([128, 8, 512], f32, tag="pp8", name="pp8", bufs=1)

        # HAM warmup: dense K=1 matmul burst during the initial DMA wait so
        # the PE clock is at 8/8 before the first real stage.  Banks are
        # reset by each stage's start=True matmuls, so garbage is harmless.
        for wi in range(16):
            nc.tensor.matmul(pp8[:, wi % 8, 0:MWMAX], ones4[0:1, 0:128],
                             ones4[0:1, 0:MWMAX], start=True, stop=True)

        # fp16 hidden states, halves adjacent at [0:mw] and [mw:2mw]
        hst = {}
        h8st = {}
        for L in (0, 1):
            for ci in range(NCH):
                for pp in (0, 1):
                    nm = f"h{L}_{ci}_{pp}"
                    hst[(L, ci, pp)] = hpool.tile([128, 2 * MWMAX], f16,
                                                  tag=nm, name=nm)
                    nm8 = f"h8_{L}_{ci}_{pp}"
                    h8st[(L, ci, pp)] = hpool.tile([128, 1024], f8,
                                                   tag=nm8, name=nm8)

        def h8_v(tile_, mw):  # [128, 2, mw] packed e4m3 view (stride 512)
            return tile_[:, :].rearrange("p (k m) -> p k m", k=2)[:, :, 0:mw]

        def wdr(w, g):  # [128, 2, 128] DR weight view for gate-half g
            return w[:, g * 256:(g + 1) * 256].rearrange("p (k f) -> p k f", k=2)

        # weight gate order: cols [z | r]; banks in region: [za zb ra rb]
        # r-banks first (they only wait sigma(i-2)); z-banks last (they also
        # wait the tanh of the cand stage that borrowed them one slot ago).
        BORDER = (2, 3, 0, 1)

        def emit_zr(nc_, t, L, ci, reg, s_zr):
            m0, mw = CHUNKS[ci]
            first = t == 0
            pp_r = 1 - t % 2

            def dr_block(w, h8t, start):
                for g in BORDER:
                    nc_.tensor.matmul(pp8[:, reg + g, 0:mw], wdr(w, g),
                                      h8_v(h8t, mw), start=start, stop=False,
                                      perf_mode=DR)

            if L == 0:
                if not first:
                    dr_block(whzr0, h8st[(0, ci, pp_r)], True)
                # row-tiled K=3 x-projection: strip 32g -> bank reg+g
                for g in BORDER:
                    sp = STRIPS[g]
                    nc_.tensor.matmul(
                        pp8[:, reg + g, 0:mw],
                        wx0p[sp:sp + 3, 0:128],
                        xtr[sp:sp + 3, t * M + m0: t * M + m0 + mw],
                        start=first, stop=True, tile_position=(sp, 0))
            else:
                h08n = h8st[(0, ci, t % 2)]
                if not first:
                    dr_block(whzr1, h8st[(1, ci, pp_r)], True)
                dr_block(wx1zr, h08n, first)
                # row-tiled K=1 bias: strip 32g -> bank reg+g
                for g in BORDER:
                    sp = STRIPS[g]
                    nc_.tensor.matmul(
                        pp8[:, reg + g, 0:mw],
                        bias1p[sp:sp + 1, 0:128],
                        ones4[sp:sp + 1, 0:mw],
                        start=False, stop=True, tile_position=(sp, 0))
            # fused sigmoid over the region's 4 banks -> s_zr [za zb ra rb]
            nc_.scalar.activation(
                s_zr[:, 0:4 * mw].rearrange("p (g m) -> p g m", g=4),
                pp8[:, reg:reg + 4, 0:mw], AF.Sigmoid)

        def emit_cand(nc_, t, L, ci, reg, s_zr, c):
            # c banks = z-banks (reg+0, reg+1) of this stage's own region
            m0, mw = CHUNKS[ci]
            first = t == 0
            pp_r = 1 - t % 2
            pp_w = t % 2
            hp = hst[(L, ci, pp_r)]
            hn = hst[(L, ci, pp_w)]
            rh = None
            if not first:
                rh = work.tile([128, 2 * MWMAX], DT.float16, tag="rh",
                               name=f"rh{L}{ci}")
                nc_.vector.tensor_mul(rh[:, 0:2 * mw], s_zr[:, 2 * mw:4 * mw],
                                      hp[:, 0:2 * mw])
            whh = whh0 if L == 0 else whh1
            if L == 0:
                # row-tiled K=3 cand x-projection: strips 0,32 -> banks reg+0,1
                for g in range(2):
                    sp = STRIPS[g]
                    nc_.tensor.matmul(
                        pp8[:, reg + g, 0:mw],
                        wx0p[sp:sp + 3, 128:256],
                        xtr[sp:sp + 3, t * M + m0: t * M + m0 + mw],
                        start=True, stop=first, tile_position=(sp, 0))
            else:
                # row-tiled K=1 cand bias (start) + Wx1c fp8 DR on h0
                for g in range(2):
                    sp = STRIPS[g]
                    nc_.tensor.matmul(
                        pp8[:, reg + g, 0:mw],
                        bias1p[sp:sp + 1, 128:256],
                        ones4[sp:sp + 1, 0:mw],
                        start=True, stop=False, tile_position=(sp, 0))
                h08n = h8st[(0, ci, pp_w)]
                for g in range(2):
                    nc_.tensor.matmul(pp8[:, reg + g, 0:mw], wdr(wx1c, g),
                                      h8_v(h08n, mw), start=False, stop=first,
                                      perf_mode=DR)
            if not first:
                for g in range(2):
                    for k in range(2):
                        nc_.tensor.matmul(
                            pp8[:, reg + g, 0:mw],
                            whh[:, k * 256 + g * 128: k * 256 + (g + 1) * 128],
                            rh[:, k * mw:(k + 1) * mw],
                            start=False, stop=(k == 1))
            # fused tanh over the 2 borrowed banks
            nc_.scalar.activation(
                c[:, 0:2 * mw].rearrange("p (g m) -> p g m", g=2),
                pp8[:, reg:reg + 2, 0:mw], AF.Tanh)
            # blend: hn = hp + z*(c - hp)
            s_z = s_zr[:, 0:2 * mw]
            if first:
                nc_.vector.tensor_mul(hn[:, 0:2 * mw], s_z, c[:, 0:2 * mw])
            else:
                d = work.tile([128, 2 * MWMAX], DT.float16, tag="d",
                              name=f"d{L}{ci}")
                nc_.vector.tensor_sub(d[:, 0:2 * mw], c[:, 0:2 * mw],
                                      hp[:, 0:2 * mw])
                zd = work.tile([128, 2 * MWMAX], DT.float16, tag="zd",
                               name=f"zd{L}{ci}")
                nc_.vector.tensor_mul(zd[:, 0:2 * mw], s_z, d[:, 0:2 * mw])
                nc_.vector.tensor_add(hn[:, 0:2 * mw], hp[:, 0:2 * mw],
                                      zd[:, 0:2 * mw])
            # packed e4m3 copy for next-step DR reads (and L1 xp for L==0);
            # dead at the last step for L==1
            if L == 0 or t < T - 1:
                # SWDGE cast-DMA keeps the e4m3 repack off the DVE; consumers
                # are >=6 stages away so the ~1us DMA latency is hidden.
                h8n = h8st[(L, ci, pp_w)]
                nc_.gpsimd.dma_start(
                    h8_v(h8n, mw),
                    hn[:, 0:2 * mw].rearrange("p (k m) -> p k m", k=2))

        stages = [(t, L, ci) for t in range(T) for L in (0, 1)
                  for ci in range(NCH)]
        pending = {}
        for si in range(len(stages) + SKEW):
            # cand first: its tanh must enter the ACT FIFO ahead of this
            # slot's sigmoid, else the next cand's z-bank matmuls stall ~1.2us
            # behind the sigmoid.
            if si >= SKEW:
                sj = si - SKEW
                t, L, cj = stages[sj]
                c = work.tile([128, 2 * MWMAX], DT.float16, tag="c",
                              name=f"c{L}{t}{cj}")
                emit_cand(nc, t, L, cj, 4 * (sj % 2), pending.pop(sj), c)
            if si < len(stages):
                t, L, ci = stages[si]
                s_zr = work.tile([128, 4 * MWMAX], DT.float16, tag="szr",
                                 name=f"szr{L}{t}{ci}", bufs=SKEW + 2)
                emit_zr(nc, t, L, ci, 4 * (si % 2), s_zr)
                pending[si] = s_zr

        ppf = (T - 1) % 2
        for L in (0, 1):
            for ci, (m0, mw) in enumerate(CHUNKS):
                nc.sync.dma_start(out_d[L, :, 2 * m0:2 * m0 + 2 * mw],
                                  hst[(L, ci, ppf)][:, 0:2 * mw])

    nc.compile()
    return nc


def _prep_weights(inputs):
    def f32(x):
        return np.asarray(x, np.float32)

    def q8c(x):
        return np.clip(f32(x), -240, 240).astype(E4M3)

    def dr_pack(W):  # (256, G*128) -> (128, G*256) DR layout
        G = W.shape[1] // 128
        out = np.zeros((128, G * 256), np.float32)
        for g in range(G):
            for k in range(2):
                out[:, g * 256 + k * 128:g * 256 + (k + 1) * 128] = \
                    W[k * 128:(k + 1) * 128, g * 128:(g + 1) * 128]
        return out

    def kstack(W):  # (256, C) -> (128, 2C)
        return np.concatenate([W[:128], W[128:]], axis=1)

    ball = {}
    for L in (0, 1):
        bx = f32(inputs[f"bx{L}"])
        bhzr = f32(inputs[f"bhzr{L}"])
        bhh = f32(inputs[f"bhh{L}"])
        ball[L] = np.concatenate([bx[:2 * H] + bhzr, bx[2 * H:] + bhh])

    # wx0p: strips of [Wx0 | b0] columns; [sp:sp+3, 0:128] = zr gate g,
    # [sp:sp+3, 128:256] = cand gate g (g=0,1)
    wx0 = np.concatenate([f32(inputs["Wx0"]), ball[0][None, :]], axis=0)
    wx0p = np.zeros((99, 256), np.float32)
    bias1p = np.zeros((99, 256), np.float32)
    for g in range(4):
        sp = STRIPS[g]
        wx0p[sp:sp + 3, 0:128] = wx0[:, g * 128:(g + 1) * 128]
        bias1p[sp, 0:128] = ball[1][g * 128:(g + 1) * 128]
    for g in range(2):
        sp = STRIPS[g]
        wx0p[sp:sp + 3, 128:256] = wx0[:, 512 + g * 128:512 + (g + 1) * 128]
        bias1p[sp, 128:256] = ball[1][512 + g * 128:512 + (g + 1) * 128]

    wx1 = f32(inputs["Wx1"])
    return {
        "wx0p": wx0p.astype(F16),
        "bias1p": bias1p.astype(F16),
        "whzr0": q8c(dr_pack(f32(inputs["Whzr0"]))),
        "whh0": kstack(f32(inputs["Whh0"])).astype(F16),
        "wx1zr": q8c(dr_pack(wx1[:, :2 * H])),
        "wx1c": q8c(dr_pack(wx1[:, 2 * H:])),
        "whzr1": q8c(dr_pack(f32(inputs["Whzr1"]))),
        "whh1": kstack(f32(inputs["Whh1"])).astype(F16),
    }


def kernel(**inputs):
    X = np.asarray(inputs["X"], np.float32)
    shared = _prep_weights(inputs)

    if "nc" not in _CACHE:
        _CACHE["nc"] = _build_nc()
    nc = _CACHE["nc"]

    in_maps = []
    ones = np.ones((1, T * M), np.float32)
    for c in range(NCORES):
        Xc = X[c * B_SH:(c + 1) * B_SH]                      # (8, T, N, D)
        xt = np.ascontiguousarray(Xc.transpose(3, 1, 0, 2)).reshape(D, T * M)
        m = dict(shared)
        m["xt"] = np.concatenate([xt, ones], axis=0).astype(F16)
        in_maps.append(m)
    _CACHE["in_maps"] = in_maps

    res = None
    for attempt in range(3):
        try:
            res = bass_utils.run_bass_kernel_spmd(nc, in_maps,
                                                  core_ids=list(range(NCORES)))
            break
        except Exception:
            if attempt == 2:
                raise
    assert res is not None

    out = np.empty((2, B, N, H), np.float32)
    for c in range(NCORES):
        arr = np.asarray(res.results[c]["out"], dtype=np.float32)  # (2,128,OUTW)
        per_core = np.empty((2, M, H), np.float32)
        for ci, (m0, mw) in enumerate(CHUNKS):
            blk = arr[:, :, 2 * m0:2 * m0 + 2 * mw].reshape(2, 128, 2, mw)
            per_core[:, m0:m0 + mw, :] = blk.transpose(0, 3, 2, 1).reshape(2, mw, H)
        out[:, c * B_SH:(c + 1) * B_SH] = per_core.reshape(2, B_SH, N, H)
    return out
